# revision 1
# baseline (speedup 1.0000x reference)
"""LocallyConnected2d (non-overlapping 3x3 patches) Trainium2 kernel.

Problem: x [B=32, Cin=128, H=96, W=96], weight [Hout=32, Wout=32, Cout=128,
Cin=128, 3, 3], bias [Hout, Wout, Cout] -> out [B, Cout, Hout, Wout].

For each of the 1024 output positions (i, j) this is an independent
[B=32, K=1152] x [K=1152, Cout=128] matmul (K = Cin*KH*KW) plus bias.

Strategy:
  - Shard the 1024 positions over 8 NeuronCores by Hout rows (4 rows =
    128 positions per core).  The weight tensor (604 MB fp32) dominates,
    and position-sharding splits it evenly with zero duplication.
  - Host-side: cast x and weight to bf16 (halves the DMA bytes, which are
    the roofline) and rearrange so every DMA descriptor is a long
    contiguous run:  per-core layouts
        wk [kp=128, pos=128, ck=9, o=128]   (bf16)
        xk [kp=128, pos=128, ck=9, b=32]    (bf16)
    where the contraction index k = c*9 + p*3 + q is split as
    k = ck*128 + kp and kp sits on SBUF partitions.
  - Per position: 9 bf16 matmuls (lhsT = w chunk [128k x 128o] STATIONARY,
    which triggers the compiler-automatic Fast Weight Load since
    NumWeights==128 and dtype!=fp32; rhs = x chunk [128k x 32b] moving)
    accumulate into PSUM [128o, 32b]; a 10th bf16 matmul
    (bias[1,128] stationary x ones[1,32] moving) adds the bias.
    Keeping the whole PE stream bf16 avoids the 4x-slow fp32 path and
    the FP32HI FWL-disable erratum (measured: 210us -> ~25us PE time).
  - 16 positions share one PSUM bank [128, 512]; one DVE copy per bank
    moves results to an SBUF staging tile; 32-position staging tiles are
    DMA'd to DRAM densely (output layout [o, pos, b], transposed to
    [b, o, i, j] on host).
  - Input DMAs ride nc.sync (HWDGE ring 0), output DMAs ride nc.scalar
    (HWDGE ring 1) so a blocked store never head-of-line blocks a
    prefetch.
"""

import numpy as np
import ml_dtypes

import concourse.bass as bass
import concourse.bacc as bacc
import concourse.mybir as mybir
import concourse.tile as tile
from concourse.bass_utils import run_bass_kernel_spmd

KH = KW = 3
B, CIN, H, W_IN = 32, 128, 96, 96
HOUT, WOUT, COUT = 32, 32, 128
NCORES = 8
IPC = HOUT // NCORES          # Hout rows per core = 4
POS = IPC * WOUT              # positions per core = 128
K = CIN * KH * KW             # 1152
CK = K // 128                 # 9 k-chunks of 128

WG = 8     # positions per weight-DMA tile
XG = 16    # positions per x-DMA tile
PG = 16    # positions per PSUM bank
SG = 32    # positions per output staging tile
WBUFS = 4  # weight pool buffers
XBUFS = 2  # x pool buffers
X_ON_ACT = False  # issue x DMAs on the scalar (ACT) HWDGE ring

BF16 = mybir.dt.bfloat16
FP32 = mybir.dt.float32

_NC_CACHE = {}


def set_config(**kw):
    g = globals()
    for k, v in kw.items():
        assert k in g, k
        g[k] = v
    _NC_CACHE.clear()


def _config_key():
    return (WG, XG, PG, SG, WBUFS, XBUFS, X_ON_ACT)


def _build_bass(repeat=1, variant="full"):
    """Build the Bass program. repeat>1 wraps the body in a dynamic loop
    (identical work each trip) so wall-clock timing can amortize the axon
    dispatch overhead: T(repeat) ~= overhead + repeat * T_kernel.
    variant: "full" | "dma" (input DMAs only) | "pe" (no input DMAs) |
    "empty" (loop overhead calibration)."""
    key = ("nc", repeat, variant, _config_key())
    if key in _NC_CACHE:
        return _NC_CACHE[key]
    nc = bacc.Bacc()
    xk = nc.declare_dram_parameter("xk", [128, POS * CK * B], BF16, isOutput=False)
    wk = nc.declare_dram_parameter("wk", [128, POS * CK * COUT], BF16, isOutput=False)
    bk = nc.declare_dram_parameter("bk", [1, POS * COUT], BF16, isOutput=False)
    out = nc.declare_dram_parameter("out", [COUT, POS * B], FP32, isOutput=True)

    XW = CK * B      # x columns per position = 288
    WW = CK * COUT   # w columns per position = 1152

    with tile.TileContext(nc) as tc:
        with (
            tc.tile_pool(name="wpool", bufs=WBUFS) as wpool,
            tc.tile_pool(name="xpool", bufs=XBUFS) as xpool,
            tc.tile_pool(name="spool", bufs=2) as spool,
            tc.tile_pool(name="cpool", bufs=1) as cpool,
            tc.tile_pool(name="ppool", bufs=4, space="PSUM") as ppool,
        ):
            ones = cpool.tile([1, B], BF16)
            nc.vector.memset(ones[:], 1.0)
            bias_t = cpool.tile([1, POS * COUT], BF16)
            nc.sync.dma_start(out=bias_t[:], in_=bk[:])

            def body():
                _emit_body(nc, tc, xk, wk, out, wpool, xpool, spool, ppool,
                           ones, bias_t, variant)

            if repeat == 1:
                body()
            else:
                with tc.For_i(0, repeat, 1):
                    body()
    nc.finalize()
    _NC_CACHE[key] = nc
    return nc


def _emit_body(nc, tc, xk, wk, out, wpool, xpool, spool, ppool, ones, bias_t,
               variant="full"):
    XW = CK * B
    WW = CK * COUT
    use_dma = variant in ("full", "dma")
    use_pe = variant in ("full", "pe")
    if variant == "empty":
        nc.vector.memset(ones[:], 1.0)
        return
    if variant == "dma":
        dummy = spool.tile([COUT, SG * B], FP32, tag="dummy")
    wt = xt = st = pt = None
    for pos in range(POS):
        il, j = divmod(pos, WOUT)
        if pos % XG == 0:
            xt = xpool.tile([128, XG * XW], BF16)
            if use_dma:
                xeng = nc.scalar if X_ON_ACT else nc.sync
                xeng.dma_start(
                    out=xt[:], in_=xk[:, pos * XW : (pos + XG) * XW]
                )
            else:
                nc.vector.memset(xt[0:1, 0:1], 0)
            if not use_pe:
                nc.vector.tensor_copy(out=dummy[0:32, 0:64], in_=xt[0:32, 0:64])
        if pos % WG == 0:
            wt = wpool.tile([128, WG * WW], BF16)
            if use_dma:
                nc.sync.dma_start(
                    out=wt[:], in_=wk[:, pos * WW : (pos + WG) * WW]
                )
            else:
                nc.vector.memset(wt[0:1, 0:1], 0)
            if not use_pe:
                nc.vector.tensor_copy(out=dummy[0:32, 64:128], in_=wt[0:32, 0:64])
        if not use_pe:
            if pos == POS - 1:
                nc.scalar.dma_start(out=out[:, 0 : SG * B], in_=dummy[:])
            continue
        if pos % SG == 0:
            st = spool.tile([COUT, SG * B], FP32)
        if pos % PG == 0:
            pt = ppool.tile([COUT, PG * B], FP32)

        xo = (pos % XG) * XW
        wo = (pos % WG) * WW
        po = (pos % PG) * B
        for ck in range(CK):
            nc.tensor.matmul(
                pt[:, po : po + B],
                wt[:, wo + ck * COUT : wo + (ck + 1) * COUT],
                xt[:, xo + ck * B : xo + (ck + 1) * B],
                start=(ck == 0),
                stop=False,
            )
        nc.tensor.matmul(
            pt[:, po : po + B],
            bias_t[0:1, pos * COUT : (pos + 1) * COUT],
            ones[:],
            start=False,
            stop=True,
        )

        if pos % PG == PG - 1:
            so = ((pos - (PG - 1)) % SG) * B
            nc.vector.tensor_copy(
                out=st[:, so : so + PG * B], in_=pt[:]
            )
        if pos % SG == SG - 1:
            q0 = (pos - (SG - 1)) * B
            nc.scalar.dma_start(
                out=out[:, q0 : q0 + SG * B], in_=st[:]
            )


def _prep_inputs(x, weight, bias):
    """Host-side cast + relayout. Returns per-core input maps."""
    xb = np.asarray(x, dtype=np.float32).astype(ml_dtypes.bfloat16)
    wb = np.asarray(weight, dtype=np.float32).astype(ml_dtypes.bfloat16)
    bb = np.asarray(bias, dtype=np.float32)

    # x: [b, c, i, p, j, q] -> [i, j, k=(c,p,q), b] -> split k -> [i,j,ck,kp,b]
    xt = (
        xb.reshape(B, CIN, HOUT, KH, WOUT, KW)
        .transpose(2, 4, 1, 3, 5, 0)
        .reshape(HOUT, WOUT, K, B)
        .reshape(HOUT, WOUT, CK, 128, B)
    )
    # w: [i, j, o, c, p, q] -> [i, j, k, o] -> [i, j, ck, kp, o]
    wt = (
        wb.transpose(0, 1, 3, 4, 5, 2)
        .reshape(HOUT, WOUT, K, COUT)
        .reshape(HOUT, WOUT, CK, 128, COUT)
    )

    in_maps = []
    for c in range(NCORES):
        i0 = c * IPC
        # -> [kp, il, j, ck, {b|o}] so each SBUF partition (kp) reads one
        # long contiguous DRAM run per DMA.
        xc = np.ascontiguousarray(
            xt[i0 : i0 + IPC].transpose(3, 0, 1, 2, 4)
        ).reshape(128, POS * CK * B)
        wc = np.ascontiguousarray(
            wt[i0 : i0 + IPC].transpose(3, 0, 1, 2, 4)
        ).reshape(128, POS * CK * COUT)
        bc = np.ascontiguousarray(bb[i0 : i0 + IPC]).reshape(1, POS * COUT).astype(ml_dtypes.bfloat16)
        in_maps.append({"xk": xc, "wk": wc, "bk": bc})
    return in_maps


def _assemble(results):
    out = np.empty((B, COUT, HOUT, WOUT), dtype=np.float32)
    for c in range(NCORES):
        r = np.asarray(results[c]["out"], dtype=np.float32)
        # [o, pos*b] -> [o, il, j, b] -> [b, o, il, j]
        out[:, :, c * IPC : (c + 1) * IPC, :] = (
            r.reshape(COUT, IPC, WOUT, B).transpose(3, 0, 1, 2)
        )
    return out


def _run(inputs, trace=False, **kw):
    in_maps = _prep_inputs(inputs["x"], inputs["weight"], inputs["bias"])
    nc = _build_bass()
    res = run_bass_kernel_spmd(nc, in_maps, list(range(NCORES)), trace=trace, **kw)
    return _assemble(res.results), res


def kernel(**inputs) -> np.ndarray:
    out, _ = _run(inputs, trace=False)
    return out


def _make_exec(nc, in_maps):
    """Build the sharded jitted executable for nc and device-resident args.
    Returns (fn, dev_args)."""
    import jax
    from jax.sharding import Mesh, PartitionSpec
    from jax.experimental.shard_map import shard_map
    from concourse import bass2jax, mybir as mb

    bass2jax.install_neuronx_cc_hook()

    partition_name = (
        nc.partition_id_tensor.name if nc.partition_id_tensor else None
    )
    in_names, out_names, out_avals, zero_outs = [], [], [], []
    for alloc in nc.m.functions[0].allocations:
        if not isinstance(alloc, mb.MemoryLocationSet):
            continue
        name = alloc.memorylocations[0].name
        if alloc.kind == "ExternalInput":
            if name != partition_name:
                in_names.append(name)
        elif alloc.kind == "ExternalOutput":
            out_names.append(name)
            shape = tuple(alloc.tensor_shape)
            dtype = mb.dt.np(alloc.dtype)
            out_avals.append(jax.core.ShapedArray(shape, dtype))
            zero_outs.append(np.zeros(shape, dtype))
    n_params = len(in_names)
    all_in_names = in_names + out_names
    if partition_name is not None:
        all_in_names = all_in_names + [partition_name]

    def _body(*args):
        operands = list(args)
        if partition_name is not None:
            operands.append(bass2jax.partition_id_tensor())
        outs = bass2jax._bass_exec_p.bind(
            *operands,
            out_avals=tuple(out_avals),
            in_names=tuple(all_in_names),
            out_names=tuple(out_names),
            lowering_input_output_aliases=(),
            sim_require_finite=True,
            sim_require_nnan=True,
            nc=nc,
        )
        return tuple(outs)

    devices = jax.devices()[:NCORES]
    mesh = Mesh(np.asarray(devices), ("core",))
    n_outs = len(out_names)
    fn = jax.jit(
        shard_map(
            _body,
            mesh=mesh,
            in_specs=(PartitionSpec("core"),) * (n_params + n_outs),
            out_specs=(PartitionSpec("core"),) * n_outs,
            check_rep=False,
        ),
        keep_unused=True,
    )
    concat_in = [
        np.concatenate([np.asarray(m[name]) for m in in_maps], axis=0)
        for name in in_names
    ]
    concat_zeros = [
        np.zeros((NCORES * z.shape[0], *z.shape[1:]), z.dtype) for z in zero_outs
    ]
    sharding = jax.sharding.NamedSharding(mesh, PartitionSpec("core"))
    dev_in = [jax.device_put(a, sharding) for a in concat_in]
    dev_zeros = [jax.device_put(a, sharding) for a in concat_zeros]
    return fn, dev_in + dev_zeros


def _timed_exec(nc, in_maps, n_iters):
    """Compile nc via the bass2jax path, keep inputs device-resident, and
    return the min wall-clock seconds over n_iters calls."""
    import time

    import jax

    fn, dev_args = _make_exec(nc, in_maps)
    # warmup (compiles)
    r = fn(*dev_args)
    jax.block_until_ready(r)
    times = []
    for _ in range(n_iters):
        t0 = time.perf_counter()
        r = fn(*dev_args)
        jax.block_until_ready(r)
        times.append(time.perf_counter() - t0)
    print(f"    raw times (ms): {[f'{t * 1e3:.2f}' for t in times]}")
    # median: the axon dispatch constant is bimodal (~60ms rare / ~100ms
    # typical), so min() is a trap; medians are tight (+-0.5ms).
    return float(np.median(times)), r


def bench(inputs, r_small=1, r_big=41, n_iters=15):
    """Estimate per-kernel HW time by differencing two repeat counts."""
    in_maps = _prep_inputs(inputs["x"], inputs["weight"], inputs["bias"])
    t_small, _ = _timed_exec(_build_bass(repeat=r_small), in_maps, n_iters)
    t_big, _ = _timed_exec(_build_bass(repeat=r_big), in_maps, n_iters)
    ns = (t_big - t_small) / (r_big - r_small) * 1e9
    print(
        f"bench: T({r_small})={t_small * 1e3:.3f} ms  T({r_big})={t_big * 1e3:.3f} ms"
        f"  -> per-kernel {ns:.0f} ns"
    )
    return ns

